# revision 40
# baseline (speedup 1.0000x reference)
"""Causal self-attention with RoPE on 8 trn2 NeuronCores (Bass/Tile).

Sharding: tensor-parallel over heads (4 heads/core) x data-parallel over
batch (B=2). Core i = b*4 + t handles batch b, heads 4t..4t+3.

Per-core dataflow (all matmuls bf16, fp32 PSUM):
  - host passes x.T [C, T] so contractions always have K on partitions
  - qk.T [512(j), T] = w_qk.T @ x.T   (lhsT = w_qk natural [c, j])
  - RoPE on q.T/k.T in [d, t] layout; per-head d-columns are permuted to
    [evens | odds] so the rotation is 32-partition-block aligned.
    Fused: uw[:, {0,1}, :] = ps * (cos|sin) in one DVE op (free-dim
    broadcast of ps), then the 32-row swap of uw[:,1] is done ON THE PE
    (matmul with a constant 128x128 permutation, written back into the
    same qk PSUM bank), then a bf16 add.  No SBUF-SBUF swap DMAs.
  - v [T, 256] natural (lhsT = x.T tile) + ones column (aug) per head
  - S.T [k, q] per head = (k.T).T @ q.T   (K = d = 64; heads of a pair run
    on disjoint PE row groups -> concurrent). Diagonal k-tiles skip the
    fully-masked column prefix (matmul N = 512-off).
  - P = exp(0.125 * S.T) on ScalarE (full groups; the dead prefix of
    diagonal tiles is exp'd too but never read); causal masking only
    multiplies the 128x128 triangle block of each diagonal k-tile.
  - y_aug.T [65, q] = v_aug.T @ P  accumulated over k tiles (AV also
    skips masked prefix columns); row 64 is the softmax denominator.
  - y.T = y.T * recip(denom)  (denoms batched, broadcast via DRAM)
  - out_partial.T [C, q] = w_proj_shard.T @ y.T ; host sums the 4 partials
    of each batch and transposes.

All DRAM tensors use p-major chunk-contiguous layouts ([128, ...] with
the partition dim outermost and per-partition runs contiguous) so every
bulk DMA moves multi-KB packets; the host does the cheap reshapes.

Schedule: a few warm-up matmuls run during the initial DMAs so the PE
HAM clock-gate opens before real work; then a driver walks the 8
attention halves (qc, hp), each split into
phase 1 (score groups + exp + mask + lag-2 AV, consuming segment/proj
blocks as evenly-spread PE filler) and phase 2 (trailing AVs + PSUM
evacuation + normalize).  Each half's first score group is emitted
BEFORE the previous half's phase 2 so the PE queue never head-of-line
blocks on trailing AVs waiting for exp.  Outputs accumulate in SBUF and
leave as one contiguous DMA per q-chunk (split in two for the final).
"""

import numpy as np
import ml_dtypes

B, T, C, H = 2, 2048, 1024, 16
HD = C // H          # 64
HPC = H // 4         # heads per core = 4
JQK = 2 * HPC * HD   # 512  (q|k columns per core)
JV = HPC * HD        # 256
N_CORES = 8
TC = 512             # q/t chunk (moving free dim)
NQC = T // TC        # 4 q-chunks
NKT = T // 128       # 16 k-tiles
NCT = C // 128       # 8 contraction tiles
VGW = 66             # v group width per head: 64 v cols + 1 ones + 1 pad
ST_G = 2             # score psum group (k-tiles per exp)

_CACHE = {}


def _build():
    import concourse.tile as tile
    from concourse import bacc, mybir
    EXP = mybir.ActivationFunctionType.Exp

    bf16 = mybir.dt.bfloat16
    f32 = mybir.dt.float32

    nc = bacc.Bacc("TRN2", target_bir_lowering=False, debug=False,
                   num_devices=N_CORES)
    xT = nc.dram_tensor("xT", [NQC, 128, NCT, TC], bf16,
                        kind="ExternalInput").ap()
    w_qk = nc.dram_tensor("w_qk", [4, 128, NCT, 128], bf16,
                          kind="ExternalInput").ap()
    w_v = nc.dram_tensor("w_v", [128, NCT, JV], bf16,
                         kind="ExternalInput").ap()
    w_pr = nc.dram_tensor("w_pr", [128, 2, C], bf16,
                          kind="ExternalInput").ap()
    cs = nc.dram_tensor("cs", [128, NQC, 2, TC], bf16,
                        kind="ExternalInput").ap()
    tri = nc.dram_tensor("tri", [128, 128], bf16, kind="ExternalInput").ap()
    swp = nc.dram_tensor("swp", [128, 128], bf16, kind="ExternalInput").ap()
    yT_out = nc.dram_tensor("yT", [NQC, 128, NCT, TC], bf16,
                            kind="ExternalOutput").ap()

    with tile.TileContext(nc) as tc:
        import contextlib
        ctx = contextlib.ExitStack()
        with ctx:
            const = ctx.enter_context(tc.tile_pool(name="const", bufs=1))
            ppool = ctx.enter_context(tc.tile_pool(name="p", bufs=4))
            ypool = ctx.enter_context(tc.tile_pool(name="ysb", bufs=4))
            rpool = ctx.enter_context(tc.tile_pool(name="r", bufs=4))
            npool = ctx.enter_context(tc.tile_pool(name="n", bufs=2))
            # single-buffered: q-chunk n+1's proj casts WAR-wait on chunk
            # n's output DMA, which completes a whole attn-half earlier
            obpool = ctx.enter_context(tc.tile_pool(name="ob", bufs=1))
            # PSUM: shared pool 3 slots x 2 banks + 2 y accumulators x 1
            # bank = 8 banks
            mm_ps = ctx.enter_context(
                tc.tile_pool(name="mmps", bufs=3, space="PSUM"))
            y_ps = ctx.enter_context(
                tc.tile_pool(name="yps", bufs=2, space="PSUM"))
            dram = ctx.enter_context(
                tc.tile_pool(name="dram", bufs=4, space="DRAM"))

            # ---- resident inputs.  Everything is p-major in DRAM so each
            # load is one fat-packet descriptor; the first-needed tensors
            # are spread across the idle engine queues. ----
            t_xT = const.tile([128, NQC, NCT, TC], bf16)
            t_wqk = const.tile([128, 4, NCT, 128], bf16)
            t_wv = const.tile([128, NCT, JV], bf16)
            t_cs = const.tile([128, NQC, 2, TC], bf16)
            t_tri = const.tile([128, 128], bf16)
            t_swp = const.tile([128, 128], bf16)
            t_wpr = const.tile([128, 2, C], bf16)
            t_warm = const.tile([128, TC], bf16)

            # head DMA plan: loads are priority-striped across the 3
            # DMA-capable queues (the aggregate HBM rate ~300 GB/s is
            # the binding constraint): the first compute's operands
            # (w_qk jt0 + xT chunk0) lead all three queues, bulk behind.
            nc.sync.dma_start(out=t_wqk[:, 0], in_=w_qk[0])       # P1
            nc.scalar.dma_start(out=t_xT[:, 0, 0:4, :], in_=xT[0][:, 0:4, :])
            nc.sync.dma_start(out=t_xT[:, 0, 4:8, :], in_=xT[0][:, 4:8, :])
            nc.gpsimd.dma_start(out=t_wqk[:, 2], in_=w_qk[2])     # P2
            nc.scalar.dma_start(out=t_cs[:, 0], in_=cs[:, 0])
            nc.gpsimd.dma_start(out=t_swp, in_=swp)
            nc.gpsimd.dma_start(out=t_wqk[:, 1], in_=w_qk[1])     # P3
            nc.gpsimd.dma_start(out=t_wqk[:, 3], in_=w_qk[3])
            nc.sync.dma_start(out=t_wv, in_=w_v)
            nc.scalar.dma_start(out=t_cs[:, 1], in_=cs[:, 1])
            nc.gpsimd.dma_start(out=t_tri, in_=tri)
            nc.sync.dma_start(out=t_xT[:, 1], in_=xT[1])          # P4
            nc.scalar.dma_start(out=t_cs[:, 2], in_=cs[:, 2])
            nc.scalar.dma_start(out=t_cs[:, 3], in_=cs[:, 3])
            nc.sync.dma_start(out=t_wpr, in_=w_pr)
            nc.sync.dma_start(out=t_xT[:, 2], in_=xT[2])
            nc.sync.dma_start(out=t_xT[:, 3], in_=xT[3])

            # all-ones stationary columns for the denominator broadcast
            # matmuls (rb = ones.T @ recip_row); the two used rows sit at
            # partitions 0 and 32 (matmul base-partition constraint)
            t_oneb = const.tile([33, 64], bf16)
            nc.vector.memset(t_warm, 0.0)
            nc.vector.memset(t_oneb, 1.0)
            # zero-init only the SECOND bank of the 3 shared PSUM slots:
            # the diagonal score groups' suffix exp reads a dead prefix
            # there before the first real write; the first bank (used by
            # the first qk psums) carries no such read, so this memset
            # adds no dependency on the critical head path.
            for s in range(3):
                pz = mm_ps.tile([128, ST_G, TC], f32, tag="mm", name="pz")
                nc.vector.memset(pz[:, 1, :], 0.0)
            # pre-zero the two denominator slots so the one-shot [33,TC]
            # reciprocal never reads unwritten rows 1..31
            for s in range(2):
                dz = npool.tile([33, TC], f32, tag="d2", name="dz")
                nc.vector.memset(dz, 1.0)

            # qk.T buffer: [128, jt, T]; jt 0..1 = q head-pairs, 2..3 = k
            t_qkT = const.tile([128, 4, T], bf16)

            # v buffer: [128(t), kt_hi, 4*66]; per head 64 v + ones + pad
            t_v = const.tile([128, NKT, 4 * VGW], bf16)
            vv = t_v.rearrange("p k (h c) -> p k h c", h=4)
            for h in range(4):
                nc.vector.memset(vv[:, :, h, 64:65], 1.0)

            # HAM warm-up: keep the PE busy while the first inputs land
            # so the clock gate is open (2.4 GHz) when real matmuls start.
            wz = y_ps.tile([128, TC], f32, tag="yps", name="warm")
            for _ in range(9):
                nc.tensor.matmul(wz, lhsT=t_warm[:, 0:128], rhs=t_warm,
                                 start=True, stop=True)

            def segment_blocks(tcn):
                """yield per-block callables for t-chunk tcn: qk blocks
                (software-pipelined rope: the PE permutation + add of
                block j are emitted inside block j+1) then 4 v blocks."""
                sl = slice(tcn * TC, (tcn + 1) * TC)
                pend = []

                def qk_front(jt):
                    ps = mm_ps.tile([128, TC], f32, tag="mm", name="psqk")
                    for ci in range(NCT):
                        nc.tensor.matmul(
                            ps,
                            lhsT=t_wqk[:, jt, ci, :],
                            rhs=t_xT[:, tcn, ci, :],
                            start=(ci == 0), stop=(ci == NCT - 1))
                    # RoPE (even/odd-split): o_e = e*cos - o*sin,
                    # o_o = o*cos + e*sin; cs rows carry the sign pattern.
                    # One fused mul (ps broadcast over the cos|sin dim).
                    uw = rpool.tile([128, 2, TC], bf16, tag="ruw")
                    nc.vector.tensor_mul(
                        out=uw, in0=ps.unsqueeze(1).broadcast_to((128, 2, TC)),
                        in1=t_cs[:, tcn])
                    pend.append((jt, ps, uw))

                def flush():
                    # 32-row block swap of uw[:,1] via the PE (constant
                    # permutation matmul back into the same PSUM bank),
                    # then a bf16 add.
                    jt, ps, uw = pend.pop(0)
                    nc.tensor.matmul(ps, lhsT=t_swp, rhs=uw[:, 1, :],
                                     start=True, stop=True)
                    nc.vector.tensor_add(out=t_qkT[:, jt, sl],
                                         in0=uw[:, 0, :], in1=ps)

                def v_block(tt):
                    ps = mm_ps.tile([128, JV], f32, tag="mm", name="psv")
                    for ci in range(NCT):
                        nc.tensor.matmul(
                            ps,
                            lhsT=t_xT[:, tt // 4, ci,
                                      (tt % 4) * 128:(tt % 4 + 1) * 128],
                            rhs=t_wv[:, ci, :],
                            start=(ci == 0), stop=(ci == NCT - 1))
                    nc.vector.tensor_copy(
                        out=vv[:, tt, :, 0:64],
                        in_=ps.rearrange("p (h c) -> p h c", h=4))

                # qk order [0,2,1,3] completes head-pair 0's q AND k
                # first so its scores (and exp) start one rope earlier
                yield (lambda: qk_front(0))
                yield (lambda: (qk_front(2), flush()))
                yield (lambda: (qk_front(1), flush()))
                yield (lambda: (qk_front(3), flush()))
                yield (lambda: (v_block(4 * tcn + 0), flush()))
                for tt in range(4 * tcn + 1, 4 * tcn + 4):
                    yield (lambda t=tt: v_block(t))

            def segment(tcn):
                for f in segment_blocks(tcn):
                    f()

            def attn_half(qc, hp, y_qc, filler=None, final=False,
                          quota=0):
                """scores+softmax+AV+normalize for q-chunk qc, head pair
                hp, split into two phases: phase 1 (a generator yielding
                after each score group) emits scores+exp+mask+lagged AV;
                phase 2 (returned closure) emits the trailing AVs, PSUM
                evacuation and normalize, and is deferred by the driver
                until the next half has been set up. The normalize uses
                a PE-free DRAM-broadcast (so the PE FIFO never waits on
                the DVE reciprocal); only the final half uses a broadcast
                matmul (PE is idle then) to cut the tail latency."""
                nkt = 4 * (qc + 1)
                qsl = slice(qc * TC, (qc + 1) * TC)
                pA = ppool.tile([128, NKT, TC], bf16, tag="pbuf")
                pB = ppool.tile([128, NKT, TC], bf16, tag="pbuf")
                yA = y_ps.tile([65, TC], f32, tag="yps")
                yB = y_ps.tile([65, TC], f32, tag="yps")

                def off_of(kt):
                    off = kt * 128 - qc * TC
                    return off if 0 <= off < TC else 0

                ngrp = (nkt + ST_G - 1) // ST_G

                def av(kt):
                    off = off_of(kt)
                    for half, (yps, p) in enumerate(((yA, pA), (yB, pB))):
                        h = 2 * hp + half
                        nc.tensor.matmul(
                            yps[:, off:TC],
                            lhsT=t_v[:, kt, h * VGW:h * VGW + 65],
                            rhs=p[:, kt, off:TC],
                            start=(kt == 0), stop=(kt == nkt - 1))

                def phase1():
                    consumed = 0
                    for gi, g in enumerate(range(ngrp)):
                        g0 = g * ST_G
                        gl = min(ST_G, nkt - g0)
                        stA = mm_ps.tile([128, ST_G, TC], f32, tag="mm")
                        stB = mm_ps.tile([128, ST_G, TC], f32, tag="mm")
                        for kg in range(gl):
                            kt = g0 + kg
                            ksl = slice(kt * 128, (kt + 1) * 128)
                            off = off_of(kt)
                            qsl_o = slice(qc * TC + off, (qc + 1) * TC)
                            nc.tensor.matmul(
                                stA[:, kg, off:TC],
                                lhsT=t_qkT[0:64, 2 + hp, ksl],
                                rhs=t_qkT[0:64, hp, qsl_o],
                                start=True, stop=True)
                            nc.tensor.matmul(
                                stB[:, kg, off:TC],
                                lhsT=t_qkT[64:128, 2 + hp, ksl],
                                rhs=t_qkT[64:128, hp, qsl_o],
                                start=True, stop=True)
                        # AV for the group 2 back, plus periodic filler
                        # to cover the PE-vs-ACT deficit
                        if gi >= 2:
                            gp = gi - 2
                            for kt in range(gp * ST_G,
                                            min(gp * ST_G + ST_G, nkt)):
                                av(kt)
                        # spread the filler budget evenly across groups;
                        # skip the first 2 groups (proj fillers depend on
                        # the previous half's normalize, needs DVE slack)
                        start = 2 if ngrp > 3 else 1
                        if filler is not None and gi >= start:
                            want = -(-quota * (gi - start + 1)
                                     // (ngrp - start))
                            while consumed < want:
                                f = next(filler, None)
                                if f is None:
                                    break
                                f()
                                consumed += 1
                        # exp the group's live column suffix in one
                        # instruction per head (suffix starts at the
                        # first tile's off; later diagonal tiles' dead
                        # prefix inside that range holds stale-but-
                        # finite PSUM and is never read by AV).
                        offm = off_of(g0)
                        nc.scalar.activation(
                            out=pA[:, g0:g0 + gl, offm:TC],
                            in_=stA[:, 0:gl, offm:TC],
                            func=EXP, scale=0.125)
                        nc.scalar.activation(
                            out=pB[:, g0:g0 + gl, offm:TC],
                            in_=stB[:, 0:gl, offm:TC],
                            func=EXP, scale=0.125)
                        # causal mask: only the 128-wide triangle block
                        # of diagonal k-tiles needs masking (prefix cols
                        # are skipped in the AV/score matmuls entirely)
                        for kg in range(gl):
                            kt = g0 + kg
                            off = kt * 128 - qc * TC
                            if 0 <= off < TC:
                                dsl = slice(off, off + 128)
                                nc.vector.tensor_mul(
                                    out=pA[:, kt, dsl],
                                    in0=pA[:, kt, dsl], in1=t_tri)
                                nc.gpsimd.tensor_mul(
                                    out=pB[:, kt, dsl],
                                    in0=pB[:, kt, dsl], in1=t_tri)
                        yield

                def phase2():
                    for kt in range(max(0, (ngrp - 2) * ST_G), nkt):
                        av(kt)
                    # denominators at partitions 0 and 32 (aligned bases)
                    # so the reciprocal covers both in one 512-element op
                    d2 = npool.tile([33, TC], f32, tag="d2")
                    r2 = npool.tile([33, TC], f32, tag="r2")
                    nc.vector.tensor_copy(out=d2[0:1, :], in_=yA[64:65, :])
                    nc.vector.tensor_copy(out=d2[32:33, :],
                                          in_=yB[64:65, :])
                    if final:
                        # shortest-latency tail: recip -> bf16 broadcast
                        # matmul -> scale straight from PSUM.  (Rows
                        # 1..31 hold garbage and are never consumed.)
                        r2b = npool.tile([33, TC], bf16, tag="r2b")
                        nc.vector.reciprocal_approx_fast(out=r2, in_=d2)
                        nc.vector.tensor_copy(out=r2b, in_=r2)
                        for half, yps in ((0, yA), (1, yB)):
                            b32 = half * 32
                            rb_ps = mm_ps.tile([64, TC], f32, tag="mm",
                                               name="psrb")
                            nc.tensor.matmul(
                                rb_ps, lhsT=t_oneb[b32:b32 + 1, :],
                                rhs=r2b[b32:b32 + 1, :],
                                start=True, stop=True)
                            rb = rpool.tile([64, TC], f32, tag="r64")
                            nc.vector.tensor_copy(out=rb, in_=rb_ps)
                            nc.vector.tensor_mul(
                                out=y_qc[half * 64:(half + 1) * 64, hp, :],
                                in0=yps[0:64, :], in1=rb)
                    else:
                        # evacuate PSUM early (bodies to bf16 SBUF), then
                        # a PE-free DRAM broadcast of the reciprocals
                        ySb = npool.tile([64, 2, TC], bf16, tag="ysb")
                        nc.vector.tensor_copy(out=ySb[:, 0, :],
                                              in_=yA[0:64, :])
                        nc.vector.tensor_copy(out=ySb[:, 1, :],
                                              in_=yB[0:64, :])
                        nc.vector.reciprocal_approx_fast(out=r2, in_=d2)
                        rd = dram.tile([2, TC], f32, tag="rd")
                        nc.sync.dma_start(out=rd[0:1, :], in_=r2[0:1, :])
                        nc.sync.dma_start(out=rd[1:2, :], in_=r2[32:33, :])
                        for half in (0, 1):
                            rb = rpool.tile([64, TC], f32, tag="r64")
                            nc.sync.dma_start(
                                out=rb,
                                in_=rd[half:half + 1, :]
                                .to_broadcast((64, TC)))
                            nc.vector.tensor_mul(
                                out=y_qc[half * 64:(half + 1) * 64, hp, :],
                                in0=ySb[:, half, :], in1=rb)
                return phase1(), phase2

            def proj_blocks(qc, y_qc):
                ob = obpool.tile([128, NCT, TC], bf16, tag="ob")
                for co in range(NCT):
                    def co_block(co=co):
                        ps = mm_ps.tile([128, TC], f32, tag="mm", name="psp")
                        for ci in range(2):
                            nc.tensor.matmul(
                                ps,
                                lhsT=t_wpr[:, ci, co * 128:(co + 1) * 128],
                                rhs=y_qc[:, ci, :],
                                start=(ci == 0), stop=(ci == 1))
                        nc.vector.tensor_copy(out=ob[:, co, :], in_=ps)
                    yield co_block
                yield (lambda: nc.sync.dma_start(out=yT_out[qc], in_=ob))

            def proj_final_blocks(qc, y_qc):
                """final chunk's proj split in two passes: pass A (the
                hp=0 partial, cast to bf16 in ob) runs as phase-1 filler
                of the last half -- its y input is ready one half
                earlier; pass B (hp=1 matmul + in-place add + output
                DMAs) is all that remains on the critical tail."""
                ob = obpool.tile([128, NCT, TC], bf16, tag="ob")

                def a_block(co):
                    ps = mm_ps.tile([128, TC], f32, tag="mm", name="pspa")
                    nc.tensor.matmul(
                        ps, lhsT=t_wpr[:, 0, co * 128:(co + 1) * 128],
                        rhs=y_qc[:, 0, :], start=True, stop=True)
                    nc.vector.tensor_copy(out=ob[:, co, :], in_=ps)

                def b_block(co):
                    ps = mm_ps.tile([128, TC], f32, tag="mm", name="pspb")
                    nc.tensor.matmul(
                        ps, lhsT=t_wpr[:, 1, co * 128:(co + 1) * 128],
                        rhs=y_qc[:, 1, :], start=True, stop=True)
                    if co % 2:
                        # GpSimd cannot read PSUM: ScalarE evacuates to
                        # bf16 SBUF, GpSimd does the add -- keeps the
                        # even-co adds on VectorE flowing in parallel
                        tmp = rpool.tile([128, TC], bf16, tag="ptmp")
                        nc.scalar.copy(out=tmp, in_=ps)
                        nc.gpsimd.tensor_add(out=ob[:, co, :],
                                             in0=ob[:, co, :], in1=tmp)
                    else:
                        nc.vector.tensor_add(out=ob[:, co, :],
                                             in0=ob[:, co, :], in1=ps)
                    if co == 3:
                        nc.sync.dma_start(out=yT_out[qc][:, 0:4, :],
                                          in_=ob[:, 0:4, :])
                    if co == 5:
                        nc.scalar.dma_start(out=yT_out[qc][:, 4:6, :],
                                            in_=ob[:, 4:6, :])
                    if co == NCT - 1:
                        nc.gpsimd.dma_start(out=yT_out[qc][:, 6:8, :],
                                            in_=ob[:, 6:8, :])
                a_list = [(lambda c=co: a_block(c)) for co in range(NCT)]
                b_list = [(lambda c=co: b_block(c)) for co in range(NCT)]
                return a_list, b_list

            y_qcs = [ypool.tile([128, 2, TC], bf16, tag="yqc",
                                name=f"yqc{q}") for q in range(NQC)]
            segment(0)
            # driver: each half gets an explicit filler list (the next
            # chunk's segment blocks for the early halves, proj blocks
            # for the late ones), consumed at an even per-group rate so
            # neither the PE nor ScalarE sees a long one-sided stretch.
            # The first score group of each half is emitted BEFORE the
            # previous half's phase 2 so its scores (and exp) never queue
            # behind trailing AVs.
            seg1 = list(segment_blocks(1))
            seg2 = list(segment_blocks(2))
            seg3 = list(segment_blocks(3))
            pj = [list(proj_blocks(q, y_qcs[q])) for q in range(3)]
            fin_a, fin_b = proj_final_blocks(NQC - 1, y_qcs[NQC - 1])
            plan = [
                (0, 0, seg1[:5]),
                (0, 1, seg1[5:]),
                (1, 0, seg2),
                (1, 1, pj[0]),
                (2, 0, seg3),
                (2, 1, pj[1]),
                (3, 0, pj[2][:4]),
                (3, 1, pj[2][4:] + fin_a),
            ]
            prev_p2 = None
            for i, (qc, hp, fills) in enumerate(plan):
                fill = iter(fills)
                p1, p2 = attn_half(qc, hp, y_qcs[qc], fill,
                                   final=(i == len(plan) - 1),
                                   quota=len(fills))
                next(p1)
                if prev_p2 is not None:
                    prev_p2()
                for _ in p1:
                    pass
                for f in fill:
                    f()
                prev_p2 = p2
            prev_p2()
            for f in fin_b:
                f()

    nc.compile()
    return nc


def _prep_inputs(x, w_qkv, w_proj, freqs_cos, freqs_sin):
    bf = ml_dtypes.bfloat16
    cos = np.asarray(freqs_cos, np.float32)   # [T, 32]
    sin = np.asarray(freqs_sin, np.float32)
    # even/odd-split RoPE: within each head, q/k columns are permuted to
    # [d0,d2,..,d62, d1,d3,..,d63]; patterns are 32-row blocks
    cos_p = np.tile(cos.T, (4, 1))                             # [128, T]
    sin_p = np.tile(np.concatenate([sin.T, -sin.T], 0), (2, 1))
    # chunk-major p-major: cs[p, tcn, {cos,sin}, j]
    cs = np.stack([cos_p.reshape(128, NQC, TC),
                   sin_p.reshape(128, NQC, TC)], axis=2).astype(bf)
    eo = np.concatenate([np.arange(0, HD, 2), np.arange(1, HD, 2)])
    # causal triangle for the 128-wide diagonal block: keep iff col >= row
    kp = np.arange(128)
    tri = (kp[None, :] >= kp[:, None]).astype(bf)   # [row k, col j]: j >= k
    # rope swap permutation: out[p] = in[p ^ 32]
    swp = np.zeros((128, 128), np.float32)
    swp[kp, kp ^ 32] = 1.0
    swp = swp.astype(bf)

    def pmaj(a, nb):
        # [nb*128, w] -> [128, nb, w] p-major contiguous
        w = a.shape[1]
        return np.ascontiguousarray(
            a.reshape(nb, 128, w).transpose(1, 0, 2))

    x = np.asarray(x, np.float32)
    w_qkv = np.asarray(w_qkv, np.float32)
    w_proj = np.asarray(w_proj, np.float32)
    in_maps = []
    # per-head even/odd column permutation for q and k blocks
    perm = np.concatenate([h * HD + eo for h in range(H)])
    wq_p = w_qkv[:, 0 * C:1 * C][:, perm]
    wk_p = w_qkv[:, 1 * C:2 * C][:, perm]
    for i in range(N_CORES):
        b, t = divmod(i, 4)
        jq = slice(t * JV, (t + 1) * JV)
        wq = wq_p[:, jq]
        wk = wk_p[:, jq]
        wv = w_qkv[:, 2 * C:3 * C][:, jq]
        xTb = x[b].T                                          # [C, T]
        xTc = np.stack([
            xTb[:, tcn * TC:(tcn + 1) * TC]
            .reshape(NCT, 128, TC).transpose(1, 0, 2)
            for tcn in range(NQC)])                  # [NQC, 128, NCT, TC]
        wqk = np.concatenate([wq, wk], axis=1)           # [C, 512]
        wqk_jt = np.stack([pmaj(wqk[:, j * 128:(j + 1) * 128], NCT)
                           for j in range(4)])           # [4, 128, 8, 128]
        in_maps.append({
            "xT": np.ascontiguousarray(xTc).astype(bf),
            "w_qk": np.ascontiguousarray(wqk_jt).astype(bf),
            "w_v": pmaj(wv, NCT).astype(bf),
            "w_pr": pmaj(w_proj[t * JV:(t + 1) * JV, :], 2).astype(bf),
            "cs": cs, "tri": tri, "swp": swp,
        })
    return in_maps


def run(inputs, trace=False):
    from concourse import bass_utils
    if "nc" not in _CACHE:
        _CACHE["nc"] = _build()
    nc = _CACHE["nc"]
    in_maps = _prep_inputs(**inputs)
    res = bass_utils.run_bass_kernel_spmd(
        nc, in_maps, core_ids=list(range(N_CORES)), trace=trace)
    out = np.empty((B, T, C), np.float32)
    for b in range(B):
        acc = res.results[b * 4]["yT"].astype(np.float32)
        for t in range(1, 4):
            acc += res.results[b * 4 + t]["yT"]
        # acc[qc, p, co, j] -> outT[co*128+p, qc*TC+j]
        out[b] = acc.transpose(2, 1, 0, 3).reshape(C, T).T
    return out, res


def kernel(**inputs):
    out, _ = run(inputs, trace=False)
    return out


# revision 45
# speedup vs baseline: 1.1865x; 1.1865x over previous
"""Causal self-attention with RoPE on 8 trn2 NeuronCores (Bass/Tile).

Sharding: tensor-parallel over heads (4 heads/core) x data-parallel over
batch (B=2). Core i = b*4 + t handles batch b, heads 4t..4t+3.

Per-core dataflow (all matmuls bf16, fp32 PSUM):
  - host passes x.T [C, T] so contractions always have K on partitions
  - qk.T [512(j), T] = w_qk.T @ x.T   (lhsT = w_qk natural [c, j])
  - RoPE on q.T/k.T in [d, t] layout; per-head d-columns are permuted to
    [evens | odds] so the rotation is 32-partition-block aligned.
    Fused: uw[:, {0,1}, :] = ps * (cos|sin) in one DVE op (free-dim
    broadcast of ps), then the 32-row swap of uw[:,1] is done ON THE PE
    (matmul with a constant 128x128 permutation, written back into the
    same qk PSUM bank), then a bf16 add.  No SBUF-SBUF swap DMAs.
  - v [T, 256] natural (lhsT = x.T tile) + ones column (aug) per head
  - S.T [k, q] per head = (k.T).T @ q.T   (K = d = 64; heads of a pair run
    on disjoint PE row groups -> concurrent). Diagonal k-tiles skip the
    fully-masked column prefix (matmul N = 512-off).
  - P = exp(0.125 * S.T) on ScalarE (full groups; the dead prefix of
    diagonal tiles is exp'd too but never read); causal masking only
    multiplies the 128x128 triangle block of each diagonal k-tile.
  - y_aug.T [65, q] = v_aug.T @ P  accumulated over k tiles (AV also
    skips masked prefix columns); row 64 is the softmax denominator.
  - y.T = y.T * recip(denom)  (denoms batched, broadcast via DRAM)
  - out_partial.T [C, q] = w_proj_shard.T @ y.T ; host sums the 4 partials
    of each batch and transposes.

All DRAM tensors use p-major chunk-contiguous layouts ([128, ...] with
the partition dim outermost and per-partition runs contiguous) so every
bulk DMA moves multi-KB packets; the host does the cheap reshapes.

Schedule: a few warm-up matmuls run during the initial DMAs so the PE
HAM clock-gate opens before real work; then a driver walks the 8
attention halves (qc, hp), each split into
phase 1 (score groups + exp + mask + lag-2 AV, consuming segment/proj
blocks as evenly-spread PE filler) and phase 2 (trailing AVs + PSUM
evacuation + normalize).  Each half's first score group is emitted
BEFORE the previous half's phase 2 so the PE queue never head-of-line
blocks on trailing AVs waiting for exp.  Outputs accumulate in SBUF and
leave as one contiguous DMA per q-chunk (split in two for the final).
"""

import numpy as np
import ml_dtypes

B, T, C, H = 2, 2048, 1024, 16
HD = C // H          # 64
HPC = H // 4         # heads per core = 4
JQK = 2 * HPC * HD   # 512  (q|k columns per core)
JV = HPC * HD        # 256
N_CORES = 8
TC = 512             # q/t chunk (moving free dim)
NQC = T // TC        # 4 q-chunks
NKT = T // 128       # 16 k-tiles
NCT = C // 128       # 8 contraction tiles
VGW = 66             # v group width per head: 64 v cols + 1 ones + 1 pad
ST_G = 2             # score psum group (k-tiles per exp)

_CACHE = {}


def _build():
    import concourse.tile as tile
    from concourse import bacc, mybir
    EXP = mybir.ActivationFunctionType.Exp

    bf16 = mybir.dt.bfloat16
    f32 = mybir.dt.float32

    nc = bacc.Bacc("TRN2", target_bir_lowering=False, debug=False,
                   num_devices=N_CORES)
    xT = nc.dram_tensor("xT", [NQC, 128, NCT, TC], bf16,
                        kind="ExternalInput").ap()
    w_qk = nc.dram_tensor("w_qk", [4, 128, NCT, 128], bf16,
                          kind="ExternalInput").ap()
    w_v = nc.dram_tensor("w_v", [128, NCT, JV], bf16,
                         kind="ExternalInput").ap()
    w_pr = nc.dram_tensor("w_pr", [128, 2, C], bf16,
                          kind="ExternalInput").ap()
    cs = nc.dram_tensor("cs", [128, NQC, 2, TC], bf16,
                        kind="ExternalInput").ap()
    tri = nc.dram_tensor("tri", [128, 128], bf16, kind="ExternalInput").ap()
    swp = nc.dram_tensor("swp", [128, 128], bf16, kind="ExternalInput").ap()
    yT_out = nc.dram_tensor("yT", [NQC, 128, NCT, TC], bf16,
                            kind="ExternalOutput").ap()

    with tile.TileContext(nc) as tc:
        import contextlib
        ctx = contextlib.ExitStack()
        with ctx:
            const = ctx.enter_context(tc.tile_pool(name="const", bufs=1))
            ppool = ctx.enter_context(tc.tile_pool(name="p", bufs=4))
            ypool = ctx.enter_context(tc.tile_pool(name="ysb", bufs=4))
            rpool = ctx.enter_context(tc.tile_pool(name="r", bufs=4))
            npool = ctx.enter_context(tc.tile_pool(name="n", bufs=2))
            # single-buffered: q-chunk n+1's proj casts WAR-wait on chunk
            # n's output DMA, which completes a whole attn-half earlier
            obpool = ctx.enter_context(tc.tile_pool(name="ob", bufs=1))
            # PSUM: shared pool 3 slots x 2 banks + 2 y accumulators x 1
            # bank = 8 banks
            mm_ps = ctx.enter_context(
                tc.tile_pool(name="mmps", bufs=3, space="PSUM"))
            y_ps = ctx.enter_context(
                tc.tile_pool(name="yps", bufs=2, space="PSUM"))
            dram = ctx.enter_context(
                tc.tile_pool(name="dram", bufs=4, space="DRAM"))

            # ---- resident inputs.  Everything is p-major in DRAM so each
            # load is one fat-packet descriptor; the first-needed tensors
            # are spread across the idle engine queues. ----
            t_xT = const.tile([128, NQC, NCT, TC], bf16)
            t_wqk = const.tile([128, 4, NCT, 128], bf16)
            t_wv = const.tile([128, NCT, JV], bf16)
            t_cs = const.tile([128, NQC, 2, TC], bf16)
            t_tri = const.tile([128, 128], bf16)
            t_swp = const.tile([128, 128], bf16)
            t_wpr = const.tile([128, 2, C], bf16)
            t_warm = const.tile([128, TC], bf16)

            # head DMA plan: loads are priority-striped across the 3
            # DMA-capable queues (the aggregate HBM rate ~300 GB/s is
            # the binding constraint): the first compute's operands
            # (w_qk jt0 + xT chunk0) lead all three queues, bulk behind.
            nc.gpsimd.dma_start(out=t_wqk[:, 0], in_=w_qk[0])     # P1
            nc.scalar.dma_start(out=t_xT[:, 0, 0:4, :], in_=xT[0][:, 0:4, :])
            nc.sync.dma_start(out=t_xT[:, 0, 4:8, :], in_=xT[0][:, 4:8, :])
            nc.gpsimd.dma_start(out=t_wqk[:, 2], in_=w_qk[2])     # P2
            nc.scalar.dma_start(out=t_cs[:, 0], in_=cs[:, 0])
            nc.gpsimd.dma_start(out=t_swp, in_=swp)
            nc.gpsimd.dma_start(out=t_wqk[:, 1], in_=w_qk[1])     # P3
            nc.gpsimd.dma_start(out=t_wqk[:, 3], in_=w_qk[3])
            nc.sync.dma_start(out=t_wv, in_=w_v)
            nc.scalar.dma_start(out=t_cs[:, 1], in_=cs[:, 1])
            nc.gpsimd.dma_start(out=t_tri, in_=tri)
            nc.sync.dma_start(out=t_xT[:, 1], in_=xT[1])          # P4
            nc.scalar.dma_start(out=t_cs[:, 2], in_=cs[:, 2])
            nc.scalar.dma_start(out=t_cs[:, 3], in_=cs[:, 3])
            nc.sync.dma_start(out=t_wpr, in_=w_pr)
            nc.sync.dma_start(out=t_xT[:, 2], in_=xT[2])
            nc.sync.dma_start(out=t_xT[:, 3], in_=xT[3])

            # all-ones stationary columns for the denominator broadcast
            # matmuls (rb = ones.T @ recip_row); the two used rows sit at
            # partitions 0 and 32 (matmul base-partition constraint)
            t_oneb = const.tile([33, 64], bf16)
            nc.vector.memset(t_warm, 0.0)
            nc.vector.memset(t_oneb, 1.0)
            # zero-init only the SECOND bank of the 3 shared PSUM slots:
            # the diagonal score groups' suffix exp reads a dead prefix
            # there before the first real write; the first bank (used by
            # the first qk psums) carries no such read, so this memset
            # adds no dependency on the critical head path.
            for s in range(3):
                pz = mm_ps.tile([128, ST_G, TC], f32, tag="mm", name="pz")
                nc.vector.memset(pz[:, 1, :], 0.0)
            # pre-zero the two denominator slots so the one-shot [33,TC]
            # reciprocal never reads unwritten rows 1..31
            for s in range(2):
                dz = npool.tile([33, TC], f32, tag="d2", name="dz")
                nc.vector.memset(dz, 1.0)

            # qk.T buffer: [128, jt, T]; jt 0..1 = q head-pairs, 2..3 = k
            t_qkT = const.tile([128, 4, T], bf16)

            # v buffer: [128(t), kt_hi, 4*66]; per head 64 v + ones + pad
            t_v = const.tile([128, NKT, 4 * VGW], bf16)
            vv = t_v.rearrange("p k (h c) -> p k h c", h=4)
            for h in range(4):
                nc.vector.memset(vv[:, :, h, 64:65], 1.0)

            # HAM warm-up: keep the PE busy while the first inputs land
            # so the clock gate is open (2.4 GHz) when real matmuls start.
            # 20 warm-ups bridge the whole input-DMA wait (~7.5-13us):
            # the first ~6 run at the cold clock, the rest warm, ending
            # just before the first operands land (~14.1us) so the real
            # matmuls start at 2.4 GHz with the HAM gate already open.
            wz = y_ps.tile([128, TC], f32, tag="yps", name="warm")
            for _ in range(20):
                nc.tensor.matmul(wz, lhsT=t_warm[:, 0:128], rhs=t_warm,
                                 start=True, stop=True)

            def segment_blocks(tcn):
                """yield per-block callables for t-chunk tcn: qk blocks
                (software-pipelined rope: the PE permutation + add of
                block j are emitted inside block j+1) then 4 v blocks."""
                sl = slice(tcn * TC, (tcn + 1) * TC)
                pend = []

                def qk_front(jt):
                    ps = mm_ps.tile([128, TC], f32, tag="mm", name="psqk")
                    for ci in range(NCT):
                        nc.tensor.matmul(
                            ps,
                            lhsT=t_wqk[:, jt, ci, :],
                            rhs=t_xT[:, tcn, ci, :],
                            start=(ci == 0), stop=(ci == NCT - 1))
                    # RoPE (even/odd-split): o_e = e*cos - o*sin,
                    # o_o = o*cos + e*sin; cs rows carry the sign pattern.
                    # One fused mul (ps broadcast over the cos|sin dim).
                    uw = rpool.tile([128, 2, TC], bf16, tag="ruw")
                    nc.vector.tensor_mul(
                        out=uw, in0=ps.unsqueeze(1).broadcast_to((128, 2, TC)),
                        in1=t_cs[:, tcn])
                    pend.append((jt, ps, uw))

                def flush():
                    # 32-row block swap of uw[:,1] via the PE (constant
                    # permutation matmul back into the same PSUM bank),
                    # then a bf16 add.
                    jt, ps, uw = pend.pop(0)
                    nc.tensor.matmul(ps, lhsT=t_swp, rhs=uw[:, 1, :],
                                     start=True, stop=True)
                    nc.vector.tensor_add(out=t_qkT[:, jt, sl],
                                         in0=uw[:, 0, :], in1=ps)

                def v_block(tt):
                    ps = mm_ps.tile([128, JV], f32, tag="mm", name="psv")
                    for ci in range(NCT):
                        nc.tensor.matmul(
                            ps,
                            lhsT=t_xT[:, tt // 4, ci,
                                      (tt % 4) * 128:(tt % 4 + 1) * 128],
                            rhs=t_wv[:, ci, :],
                            start=(ci == 0), stop=(ci == NCT - 1))
                    nc.vector.tensor_copy(
                        out=vv[:, tt, :, 0:64],
                        in_=ps.rearrange("p (h c) -> p h c", h=4))

                # qk order [0,2,1,3] completes head-pair 0's q AND k
                # first so its scores (and exp) start one rope earlier
                yield (lambda: qk_front(0))
                yield (lambda: (qk_front(2), flush()))
                yield (lambda: (qk_front(1), flush()))
                yield (lambda: (qk_front(3), flush()))
                yield (lambda: (v_block(4 * tcn + 0), flush()))
                for tt in range(4 * tcn + 1, 4 * tcn + 4):
                    yield (lambda t=tt: v_block(t))

            def segment(tcn):
                for f in segment_blocks(tcn):
                    f()

            def attn_half(qc, hp, y_qc, filler=None, final=False,
                          quota=0):
                """scores+softmax+AV+normalize for q-chunk qc, head pair
                hp, split into two phases: phase 1 (a generator yielding
                after each score group) emits scores+exp+mask+lagged AV;
                phase 2 (returned closure) emits the trailing AVs, PSUM
                evacuation and normalize, and is deferred by the driver
                until the next half has been set up. The normalize uses
                a PE-free DRAM-broadcast (so the PE FIFO never waits on
                the DVE reciprocal); only the final half uses a broadcast
                matmul (PE is idle then) to cut the tail latency."""
                nkt = 4 * (qc + 1)
                qsl = slice(qc * TC, (qc + 1) * TC)
                pA = ppool.tile([128, NKT, TC], bf16, tag="pbuf")
                pB = ppool.tile([128, NKT, TC], bf16, tag="pbuf")
                yA = y_ps.tile([65, TC], f32, tag="yps")
                yB = y_ps.tile([65, TC], f32, tag="yps")

                def off_of(kt):
                    off = kt * 128 - qc * TC
                    return off if 0 <= off < TC else 0

                ngrp = (nkt + ST_G - 1) // ST_G

                def av(kt):
                    off = off_of(kt)
                    for half, (yps, p) in enumerate(((yA, pA), (yB, pB))):
                        h = 2 * hp + half
                        nc.tensor.matmul(
                            yps[:, off:TC],
                            lhsT=t_v[:, kt, h * VGW:h * VGW + 65],
                            rhs=p[:, kt, off:TC],
                            start=(kt == 0), stop=(kt == nkt - 1))

                def phase1():
                    consumed = 0
                    for gi, g in enumerate(range(ngrp)):
                        g0 = g * ST_G
                        gl = min(ST_G, nkt - g0)
                        stA = mm_ps.tile([128, ST_G, TC], f32, tag="mm")
                        stB = mm_ps.tile([128, ST_G, TC], f32, tag="mm")
                        for kg in range(gl):
                            kt = g0 + kg
                            ksl = slice(kt * 128, (kt + 1) * 128)
                            off = off_of(kt)
                            qsl_o = slice(qc * TC + off, (qc + 1) * TC)
                            nc.tensor.matmul(
                                stA[:, kg, off:TC],
                                lhsT=t_qkT[0:64, 2 + hp, ksl],
                                rhs=t_qkT[0:64, hp, qsl_o],
                                start=True, stop=True)
                            nc.tensor.matmul(
                                stB[:, kg, off:TC],
                                lhsT=t_qkT[64:128, 2 + hp, ksl],
                                rhs=t_qkT[64:128, hp, qsl_o],
                                start=True, stop=True)
                        # AV for the group 2 back, plus periodic filler
                        # to cover the PE-vs-ACT deficit
                        if gi >= 2:
                            gp = gi - 2
                            for kt in range(gp * ST_G,
                                            min(gp * ST_G + ST_G, nkt)):
                                av(kt)
                        # spread the filler budget evenly across groups;
                        # skip the first 2 groups (proj fillers depend on
                        # the previous half's normalize, needs DVE slack)
                        start = 2 if ngrp > 3 else 1
                        if filler is not None and gi >= start:
                            want = -(-quota * (gi - start + 1)
                                     // (ngrp - start))
                            while consumed < want:
                                f = next(filler, None)
                                if f is None:
                                    break
                                f()
                                consumed += 1
                        # exp the group's live column suffix in one
                        # instruction per head (suffix starts at the
                        # first tile's off; later diagonal tiles' dead
                        # prefix inside that range holds stale-but-
                        # finite PSUM and is never read by AV).
                        offm = off_of(g0)
                        nc.scalar.activation(
                            out=pA[:, g0:g0 + gl, offm:TC],
                            in_=stA[:, 0:gl, offm:TC],
                            func=EXP, scale=0.125)
                        nc.scalar.activation(
                            out=pB[:, g0:g0 + gl, offm:TC],
                            in_=stB[:, 0:gl, offm:TC],
                            func=EXP, scale=0.125)
                        # causal mask: only the 128-wide triangle block
                        # of diagonal k-tiles needs masking (prefix cols
                        # are skipped in the AV/score matmuls entirely)
                        for kg in range(gl):
                            kt = g0 + kg
                            off = kt * 128 - qc * TC
                            if 0 <= off < TC:
                                dsl = slice(off, off + 128)
                                nc.vector.tensor_mul(
                                    out=pA[:, kt, dsl],
                                    in0=pA[:, kt, dsl], in1=t_tri)
                                nc.gpsimd.tensor_mul(
                                    out=pB[:, kt, dsl],
                                    in0=pB[:, kt, dsl], in1=t_tri)
                        yield

                def phase2():
                    for kt in range(max(0, (ngrp - 2) * ST_G), nkt):
                        av(kt)
                    # denominators at partitions 0 and 32 (aligned bases)
                    # so the reciprocal covers both in one 512-element op
                    d2 = npool.tile([33, TC], f32, tag="d2")
                    r2 = npool.tile([33, TC], f32, tag="r2")
                    nc.vector.tensor_copy(out=d2[0:1, :], in_=yA[64:65, :])
                    nc.vector.tensor_copy(out=d2[32:33, :],
                                          in_=yB[64:65, :])
                    if final:
                        # shortest-latency tail: recip -> bf16 broadcast
                        # matmul -> scale straight from PSUM.  (Rows
                        # 1..31 hold garbage and are never consumed.)
                        r2b = npool.tile([33, TC], bf16, tag="r2b")
                        nc.vector.reciprocal_approx_fast(out=r2, in_=d2)
                        nc.vector.tensor_copy(out=r2b, in_=r2)
                        for half, yps in ((0, yA), (1, yB)):
                            b32 = half * 32
                            rb_ps = mm_ps.tile([64, TC], f32, tag="mm",
                                               name="psrb")
                            nc.tensor.matmul(
                                rb_ps, lhsT=t_oneb[b32:b32 + 1, :],
                                rhs=r2b[b32:b32 + 1, :],
                                start=True, stop=True)
                            rb = rpool.tile([64, TC], f32, tag="r64")
                            nc.vector.tensor_copy(out=rb, in_=rb_ps)
                            nc.vector.tensor_mul(
                                out=y_qc[half * 64:(half + 1) * 64, hp, :],
                                in0=yps[0:64, :], in1=rb)
                    else:
                        # evacuate PSUM early (bodies to bf16 SBUF), then
                        # a PE-free DRAM broadcast of the reciprocals
                        ySb = npool.tile([64, 2, TC], bf16, tag="ysb")
                        nc.vector.tensor_copy(out=ySb[:, 0, :],
                                              in_=yA[0:64, :])
                        nc.vector.tensor_copy(out=ySb[:, 1, :],
                                              in_=yB[0:64, :])
                        nc.vector.reciprocal_approx_fast(out=r2, in_=d2)
                        rd = dram.tile([2, TC], f32, tag="rd")
                        nc.sync.dma_start(out=rd[0:1, :], in_=r2[0:1, :])
                        nc.sync.dma_start(out=rd[1:2, :], in_=r2[32:33, :])
                        for half in (0, 1):
                            rb = rpool.tile([64, TC], f32, tag="r64")
                            nc.sync.dma_start(
                                out=rb,
                                in_=rd[half:half + 1, :]
                                .to_broadcast((64, TC)))
                            nc.vector.tensor_mul(
                                out=y_qc[half * 64:(half + 1) * 64, hp, :],
                                in0=ySb[:, half, :], in1=rb)
                return phase1(), phase2

            def proj_blocks(qc, y_qc, final=False):
                ob = obpool.tile([128, NCT, TC], bf16, tag="ob")
                for co in range(NCT):
                    def co_block(co=co):
                        ps = mm_ps.tile([128, TC], f32, tag="mm", name="psp")
                        for ci in range(2):
                            nc.tensor.matmul(
                                ps,
                                lhsT=t_wpr[:, ci, co * 128:(co + 1) * 128],
                                rhs=y_qc[:, ci, :],
                                start=(ci == 0), stop=(ci == 1))
                        # in the final chunk ScalarE is done with exp, so
                        # split the PSUM->SBUF casts across both engines
                        # to shorten the tail
                        if final and co % 2 == 1:
                            nc.scalar.copy(out=ob[:, co, :], in_=ps)
                        else:
                            nc.vector.tensor_copy(out=ob[:, co, :], in_=ps)
                        if final and co == 3:
                            nc.sync.dma_start(out=yT_out[qc][:, 0:4, :],
                                              in_=ob[:, 0:4, :])
                        if final and co == 5:
                            nc.scalar.dma_start(out=yT_out[qc][:, 4:6, :],
                                                in_=ob[:, 4:6, :])
                        if final and co == NCT - 1:
                            nc.gpsimd.dma_start(out=yT_out[qc][:, 6:8, :],
                                                in_=ob[:, 6:8, :])
                    yield co_block
                if not final:
                    yield (lambda: nc.sync.dma_start(out=yT_out[qc], in_=ob))

            y_qcs = [ypool.tile([128, 2, TC], bf16, tag="yqc",
                                name=f"yqc{q}") for q in range(NQC)]
            segment(0)
            # driver: each half gets an explicit filler list (the next
            # chunk's segment blocks for the early halves, proj blocks
            # for the late ones), consumed at an even per-group rate so
            # neither the PE nor ScalarE sees a long one-sided stretch.
            # The first score group of each half is emitted BEFORE the
            # previous half's phase 2 so its scores (and exp) never queue
            # behind trailing AVs.
            seg1 = list(segment_blocks(1))
            seg2 = list(segment_blocks(2))
            seg3 = list(segment_blocks(3))
            pj = [list(proj_blocks(q, y_qcs[q])) for q in range(3)]
            plan = [
                (0, 0, seg1[:5]),
                (0, 1, seg1[5:]),
                (1, 0, seg2),
                (1, 1, pj[0]),
                (2, 0, seg3),
                (2, 1, pj[1]),
                (3, 0, pj[2][:4]),
                (3, 1, pj[2][4:]),
            ]
            prev_p2 = None
            for i, (qc, hp, fills) in enumerate(plan):
                fill = iter(fills)
                p1, p2 = attn_half(qc, hp, y_qcs[qc], fill,
                                   final=(i == len(plan) - 1),
                                   quota=len(fills))
                next(p1)
                if prev_p2 is not None:
                    prev_p2()
                for _ in p1:
                    pass
                for f in fill:
                    f()
                prev_p2 = p2
            prev_p2()
            for f in proj_blocks(NQC - 1, y_qcs[NQC - 1], final=True):
                f()

    nc.compile()
    return nc


def _prep_inputs(x, w_qkv, w_proj, freqs_cos, freqs_sin):
    bf = ml_dtypes.bfloat16
    cos = np.asarray(freqs_cos, np.float32)   # [T, 32]
    sin = np.asarray(freqs_sin, np.float32)
    # even/odd-split RoPE: within each head, q/k columns are permuted to
    # [d0,d2,..,d62, d1,d3,..,d63]; patterns are 32-row blocks
    cos_p = np.tile(cos.T, (4, 1))                             # [128, T]
    sin_p = np.tile(np.concatenate([sin.T, -sin.T], 0), (2, 1))
    # chunk-major p-major: cs[p, tcn, {cos,sin}, j]
    cs = np.stack([cos_p.reshape(128, NQC, TC),
                   sin_p.reshape(128, NQC, TC)], axis=2).astype(bf)
    eo = np.concatenate([np.arange(0, HD, 2), np.arange(1, HD, 2)])
    # causal triangle for the 128-wide diagonal block: keep iff col >= row
    kp = np.arange(128)
    tri = (kp[None, :] >= kp[:, None]).astype(bf)   # [row k, col j]: j >= k
    # rope swap permutation: out[p] = in[p ^ 32]
    swp = np.zeros((128, 128), np.float32)
    swp[kp, kp ^ 32] = 1.0
    swp = swp.astype(bf)

    def pmaj(a, nb):
        # [nb*128, w] -> [128, nb, w] p-major contiguous
        w = a.shape[1]
        return np.ascontiguousarray(
            a.reshape(nb, 128, w).transpose(1, 0, 2))

    x = np.asarray(x, np.float32)
    w_qkv = np.asarray(w_qkv, np.float32)
    w_proj = np.asarray(w_proj, np.float32)
    in_maps = []
    # per-head even/odd column permutation for q and k blocks
    perm = np.concatenate([h * HD + eo for h in range(H)])
    wq_p = w_qkv[:, 0 * C:1 * C][:, perm]
    wk_p = w_qkv[:, 1 * C:2 * C][:, perm]
    for i in range(N_CORES):
        b, t = divmod(i, 4)
        jq = slice(t * JV, (t + 1) * JV)
        wq = wq_p[:, jq]
        wk = wk_p[:, jq]
        wv = w_qkv[:, 2 * C:3 * C][:, jq]
        xTb = x[b].T                                          # [C, T]
        xTc = np.stack([
            xTb[:, tcn * TC:(tcn + 1) * TC]
            .reshape(NCT, 128, TC).transpose(1, 0, 2)
            for tcn in range(NQC)])                  # [NQC, 128, NCT, TC]
        wqk = np.concatenate([wq, wk], axis=1)           # [C, 512]
        wqk_jt = np.stack([pmaj(wqk[:, j * 128:(j + 1) * 128], NCT)
                           for j in range(4)])           # [4, 128, 8, 128]
        in_maps.append({
            "xT": np.ascontiguousarray(xTc).astype(bf),
            "w_qk": np.ascontiguousarray(wqk_jt).astype(bf),
            "w_v": pmaj(wv, NCT).astype(bf),
            "w_pr": pmaj(w_proj[t * JV:(t + 1) * JV, :], 2).astype(bf),
            "cs": cs, "tri": tri, "swp": swp,
        })
    return in_maps


def run(inputs, trace=False):
    from concourse import bass_utils
    if "nc" not in _CACHE:
        _CACHE["nc"] = _build()
    nc = _CACHE["nc"]
    in_maps = _prep_inputs(**inputs)
    res = bass_utils.run_bass_kernel_spmd(
        nc, in_maps, core_ids=list(range(N_CORES)), trace=trace)
    out = np.empty((B, T, C), np.float32)
    for b in range(B):
        acc = res.results[b * 4]["yT"].astype(np.float32)
        for t in range(1, 4):
            acc += res.results[b * 4 + t]["yT"]
        # acc[qc, p, co, j] -> outT[co*128+p, qc*TC+j]
        out[b] = acc.transpose(2, 1, 0, 3).reshape(C, T).T
    return out, res


def kernel(**inputs):
    out, _ = run(inputs, trace=False)
    return out
